# revision 3
# baseline (speedup 1.0000x reference)
"""Trainium2 Bass kernel for the Gaussian-bump decoder (nn_Decoder).

Math (per (bt, n) with bt = b*T + t):
  torus ensemble (K=1, L=1):
    g0   = exp(-((sin z0 - sin m0 + sin rf0_n)^2 + (cos z0 - cos m0 + cos rf0_n)^2)/s0^2)
    r0   = exp(coeff0 * g0)
  euclidean ensemble (K=16 on a separable 4x4 grid, L=2):
    U_i  = exp(-(z1_0 + rf1_n0 - p_i)^2 / s^2)
    V_j  = exp(-(z1_1 + rf1_n1 - p_j)^2 / s^2)
    resp = sum_ij C_ij U_i V_j
    r1   = exp(resp)
  out  = (w0_n r0 + w1_n r1) * exp(lfs_n)   -> returned as (B, N, T)

Device mapping (per core, neurons sharded 8 ways: 128 train + 32 test):
  - All exp-args are tiny-contraction fp16 matmuls (hi/lo split => ~fp32
    accuracy) with the z-derived "moving" rows built on host.
  - U/V tiles are packed [128 = 4i x 32n, bt-chunk]; the 4x4 coefficient
    contraction (W = C V) and the i-sum are fp16 matmuls with block
    patterns; the softmax weights / lfs fold into the exp biases.
"""

import numpy as np

import concourse.bass as bass
import concourse.tile as tile
from concourse import bacc, mybir
from concourse.bass_utils import run_bass_kernel_spmd

F32 = mybir.dt.float32
F16 = mybir.dt.float16
EXP = mybir.ActivationFunctionType.Exp
MULT = mybir.AluOpType.mult
ADD = mybir.AluOpType.add

B, T = 16, 256
NTR, NTE = 1024, 256
BT = B * T                     # 4096
NCORES = 8
NTRL = NTR // NCORES           # 128 train neurons per core
NTEL = NTE // NCORES           # 32 test neurons per core
GK = 4                         # grid side (K = GK*GK)
CHUNK = 512
NCHUNK = BT // CHUNK           # 8
NQUAD = NCHUNK // 4            # 2 packed quads for the test ensemble

_DRAM_SPECS = [
    ("m1u", [7, BT], F16), ("m1v", [7, BT], F16), ("m0", [8, BT], F16),
    ("cbdh", [128, 128], F16), ("cbdl", [128, 128], F16),
    ("sumpat", [128, 512], F16),
    ("lu_tr", [7, 512], F16), ("lv_tr", [7, 512], F16),
    ("lu_te", [7, 128], F16), ("lv_te", [7, 128], F16),
    ("ltor_tr", [8, 128], F16), ("ltor_te", [8, 512], F16),
    ("b_tr1", [128, 1], F32), ("b_tr0", [128, 1], F32),
    ("b_te1", [128, 1], F32), ("b_te0", [128, 1], F32),
    ("c0s", [128, 1], F32),
]


def _emit(tc, d):
    nc = tc.nc
    with (
        tc.tile_pool(name="consts", bufs=1) as cp,
        tc.tile_pool(name="pu", bufs=1, space="PSUM") as pup,
        tc.tile_pool(name="pv", bufs=1, space="PSUM") as pvp,
        tc.tile_pool(name="pw", bufs=2, space="PSUM") as pwp,
        tc.tile_pool(name="presp", bufs=1, space="PSUM") as prp,
        tc.tile_pool(name="pg0", bufs=1, space="PSUM") as pgp,
        tc.tile_pool(name="pte", bufs=1, space="PSUM") as ptep,
        tc.tile_pool(name="sb", bufs=3) as sp,
        tc.tile_pool(name="outp", bufs=3) as op_,
    ):
        # ---- load constants ----
        ct = {}
        for name, shape, dt in _DRAM_SPECS:
            ct[name] = cp.tile(shape, dt, tag=name, name=name)
            nc.sync.dma_start(ct[name][:], d[name][:])

        def act(out_ap, in_ap, scale=1.0, bias=0.0):
            nc.scalar.activation(out_ap, in_ap, EXP, scale=scale, bias=bias)

        pte_resp = pte_g0 = None
        for c in range(NCHUNK):
            sl = bass.ts(c, CHUNK)
            q, a = divmod(c, 4)

            # ---------------- train: euclidean ensemble ----------------
            presp = prp.tile([128, CHUNK], F32, tag="presp")
            for blk in range(4):
                bsl = bass.ts(blk, 128)
                pu = pup.tile([128, CHUNK], F32, tag="pu")
                nc.tensor.matmul(pu[:], ct["lu_tr"][:, bsl], ct["m1u"][:, sl],
                                 start=True, stop=True)
                usb = sp.tile([128, CHUNK], F32, tag="usb")
                act(usb[:], pu[:])
                pv = pvp.tile([128, CHUNK], F32, tag="pv")
                nc.tensor.matmul(pv[:], ct["lv_tr"][:, bsl], ct["m1v"][:, sl],
                                 start=True, stop=True)
                vsb = sp.tile([128, CHUNK], F16, tag="vsb")
                act(vsb[:], pv[:])
                pw = pwp.tile([128, CHUNK], F32, tag="pw")
                nc.tensor.matmul(pw[:], ct["cbdh"][:], vsb[:], start=True, stop=False)
                nc.tensor.matmul(pw[:], ct["cbdl"][:], vsb[:], start=False, stop=True)
                ssb = sp.tile([128, CHUNK], F16, tag="ssb")
                nc.vector.tensor_tensor(ssb[:], usb[:], pw[:], op=MULT)
                nc.tensor.matmul(presp[:], ct["sumpat"][:, bsl], ssb[:],
                                 start=(blk == 0), stop=(blk == 3))
            r1 = sp.tile([128, CHUNK], F32, tag="r1")
            act(r1[:], presp[:], bias=ct["b_tr1"][:, 0:1])

            # ---------------- train: torus ensemble ----------------
            pg0 = pgp.tile([128, CHUNK], F32, tag="pg0")
            nc.tensor.matmul(pg0[:], ct["ltor_tr"][:], ct["m0"][:, sl],
                             start=True, stop=True)
            g0 = sp.tile([128, CHUNK], F32, tag="g0")
            act(g0[:], pg0[:])
            r0 = sp.tile([128, CHUNK], F32, tag="r0")
            act(r0[:], g0[:], scale=ct["c0s"][:, 0:1], bias=ct["b_tr0"][:, 0:1])

            otr = op_.tile([128, CHUNK], F32, tag="otr")
            nc.vector.tensor_tensor(otr[:], r0[:], r1[:], op=ADD)
            nc.sync.dma_start(d["out_tr"][:, sl], otr[:])

            # ---------------- test (chunk a of quad q) ----------------
            if a == 0:
                pte_resp = ptep.tile([128, CHUNK], F32, tag="pte_resp")
                pte_g0 = ptep.tile([128, CHUNK], F32, tag="pte_g0")
            pu = pup.tile([128, CHUNK], F32, tag="pu")
            nc.tensor.matmul(pu[:], ct["lu_te"][:], ct["m1u"][:, sl],
                             start=True, stop=True)
            usb = sp.tile([128, CHUNK], F32, tag="usb")
            act(usb[:], pu[:])
            pv = pvp.tile([128, CHUNK], F32, tag="pv")
            nc.tensor.matmul(pv[:], ct["lv_te"][:], ct["m1v"][:, sl],
                             start=True, stop=True)
            vsb = sp.tile([128, CHUNK], F16, tag="vsb")
            act(vsb[:], pv[:])
            pw = pwp.tile([128, CHUNK], F32, tag="pw")
            nc.tensor.matmul(pw[:], ct["cbdh"][:], vsb[:], start=True, stop=False)
            nc.tensor.matmul(pw[:], ct["cbdl"][:], vsb[:], start=False, stop=True)
            ssb = sp.tile([128, CHUNK], F16, tag="ssb")
            nc.vector.tensor_tensor(ssb[:], usb[:], pw[:], op=MULT)
            asl = bass.ts(a, 128)
            nc.tensor.matmul(pte_resp[:], ct["sumpat"][:, asl], ssb[:],
                             start=(a == 0), stop=(a == 3))
            nc.tensor.matmul(pte_g0[:], ct["ltor_te"][:, asl], ct["m0"][:, sl],
                             start=(a == 0), stop=(a == 3))
            if a == 3:
                r1te = sp.tile([128, CHUNK], F32, tag="r1te")
                act(r1te[:], pte_resp[:], bias=ct["b_te1"][:, 0:1])
                g0te = sp.tile([128, CHUNK], F32, tag="g0te")
                act(g0te[:], pte_g0[:])
                r0te = sp.tile([128, CHUNK], F32, tag="r0te")
                act(r0te[:], g0te[:], scale=ct["c0s"][:, 0:1], bias=ct["b_te0"][:, 0:1])
                ote = op_.tile([128, CHUNK], F32, tag="ote")
                nc.vector.tensor_tensor(ote[:], r0te[:], r1te[:], op=ADD)
                nc.sync.dma_start(d["out_te"][:, bass.ts(q, CHUNK)], ote[:])


_CACHE = {}


def _build():
    if "nc" in _CACHE:
        return _CACHE["nc"]
    nc = bacc.Bacc("TRN2", target_bir_lowering=False, debug=False,
                   num_devices=NCORES)
    d = {}
    for name, shape, dt in _DRAM_SPECS:
        d[name] = nc.dram_tensor(name, shape, dt, kind="ExternalInput").ap()
    d["out_tr"] = nc.dram_tensor("out_tr", [NTRL, BT], F32,
                                 kind="ExternalOutput").ap()
    d["out_te"] = nc.dram_tensor("out_te", [128, NQUAD * CHUNK], F32,
                                 kind="ExternalOutput").ap()
    with tile.TileContext(nc) as tc:
        _emit(tc, d)
    nc.compile()
    _CACHE["nc"] = nc
    return nc


def _hl(x):
    """fp32 -> (hi, lo) fp16 split."""
    x = np.asarray(x, np.float32)
    h = x.astype(np.float16)
    lo = (x - h.astype(np.float32)).astype(np.float16)
    return h, lo


def _arg_consts(rf_col, pts, s2):
    """lhsT columns for  -(z + rf - p_i)^2 / s2  per (neuron, i).

    Returns [7, N, 4] fp16: rows pair with moving rows
    [qh, ql, zh, zl, zh, 1, 1] where q = -z^2/s2.
    """
    c = rf_col[:, None] - pts[None, :]          # (N, 4) fp64
    g = np.float32(-2.0 * c / s2)
    b = np.float32(-(c * c) / s2)
    gh, gl = _hl(g)
    bh, bl = _hl(b)
    ones = np.ones_like(gh)
    return np.stack([ones, ones, gh, gh, gl, bh, bl]).astype(np.float16)


def _moving_rows(z_col, s2):
    """Moving rows [qh, ql, zh, zl, zh, 1, 1] with q = -z^2/s2."""
    q = np.float32(-(z_col.astype(np.float64) ** 2) / s2)
    qh, ql = _hl(q)
    zh, zl = _hl(np.float32(z_col))
    ones = np.ones(z_col.shape[0], np.float32)
    return np.stack([qh, ql, zh, zl, zh, ones, ones]).astype(np.float16)


def _prepare(inputs):
    f64 = np.float64
    z0 = np.asarray(inputs["z0"]).reshape(BT).astype(f64)
    z1 = np.asarray(inputs["z1"]).reshape(BT, 2).astype(f64)
    coeff0 = f64(np.asarray(inputs["coeff0"])[0, 0])
    mean0 = f64(np.asarray(inputs["mean0"]).reshape(-1)[0])
    log_var0 = f64(np.asarray(inputs["log_var0"]).reshape(-1)[0])
    C = np.asarray(inputs["coeff1"]).reshape(GK, GK).astype(f64)
    mean1 = np.asarray(inputs["mean1"]).reshape(GK * GK, 2).astype(f64)
    log_var1 = np.asarray(inputs["log_var1"]).reshape(GK * GK, 2).astype(f64)
    # separable grid: mean1[(i,j), 0] = pts[i], mean1[(i,j), 1] = pts[j]
    pts0 = mean1.reshape(GK, GK, 2)[:, 0, 0]
    pts1 = mean1.reshape(GK, GK, 2)[0, :, 1]
    s2u = f64(np.exp(log_var1[0, 0])) ** 2
    s2v = f64(np.exp(log_var1[0, 1])) ** 2
    s20 = f64(np.exp(log_var0)) ** 2

    # shared moving operands
    m1u = _moving_rows(z1[:, 0], s2u)
    m1v = _moving_rows(z1[:, 1], s2v)
    as_ = np.float32(np.sin(z0) - np.sin(mean0))
    ac_ = np.float32(np.cos(z0) - np.cos(mean0))
    p = np.float32(-(as_.astype(f64) ** 2 + ac_.astype(f64) ** 2 + 1.0) / s20)
    ph, pl = _hl(p)
    ash, asl = _hl(as_)
    ach, acl = _hl(ac_)
    m0 = np.stack([ph, pl, ash, asl, ash, ach, acl, ach]).astype(np.float16)

    # C block-diagonal (4x4 per 32-neuron diag block), partitions i-major:
    # row p_in = j*32 + n32 ; col p_out = i*32 + n32 ; value C[i, j]
    Ch, Cl = _hl(np.float32(C))
    cbdh = np.zeros((128, 128), np.float16)
    cbdl = np.zeros((128, 128), np.float16)
    for i in range(GK):
        for j in range(GK):
            idx = np.arange(32)
            cbdh[j * 32 + idx, i * 32 + idx] = Ch[i, j]
            cbdl[j * 32 + idx, i * 32 + idx] = Cl[i, j]
    # i-sum patterns: block b: lhsT[p, col] = 1 iff col == 32*b + (p % 32)
    sumpat = np.zeros((128, 512), np.float16)
    for b in range(4):
        for i in range(GK):
            idx = np.arange(32)
            sumpat[i * 32 + idx, 128 * b + 32 * b + idx] = 1.0
    shared = dict(m1u=m1u, m1v=m1v, m0=m0, cbdh=cbdh, cbdl=cbdl, sumpat=sumpat)

    rf_tr0 = np.asarray(inputs["rf_tr0"]).reshape(NTR).astype(f64)
    rf_te0 = np.asarray(inputs["rf_te0"]).reshape(NTE).astype(f64)
    rf_tr1 = np.asarray(inputs["rf_tr1"]).astype(f64)
    rf_te1 = np.asarray(inputs["rf_te1"]).astype(f64)
    lu_tr_all = _arg_consts(rf_tr1[:, 0], pts0, s2u)   # [7, NTR, 4]
    lv_tr_all = _arg_consts(rf_tr1[:, 1], pts1, s2v)
    lu_te_all = _arg_consts(rf_te1[:, 0], pts0, s2u)
    lv_te_all = _arg_consts(rf_te1[:, 1], pts1, s2v)

    def torus_consts(rf0):
        bs = np.float32(np.sin(rf0))
        bc = np.float32(np.cos(rf0))
        gs = np.float32(-2.0 * bs.astype(f64) / s20)
        gc = np.float32(-2.0 * bc.astype(f64) / s20)
        gsh, gsl = _hl(gs)
        gch, gcl = _hl(gc)
        ones = np.ones_like(gsh)
        return np.stack([ones, ones, gsh, gsh, gsl, gch, gch, gcl]).astype(np.float16)

    tor_tr_all = torus_consts(rf_tr0)   # [8, NTR]
    tor_te_all = torus_consts(rf_te0)   # [8, NTE]

    def softmax_ln(ew):
        ew = np.asarray(ew).astype(f64)
        m = ew.max(axis=1, keepdims=True)
        e = np.exp(ew - m)
        w = e / e.sum(axis=1, keepdims=True)
        return np.log(w[:, 0]), np.log(w[:, 1])

    lnw0_tr, lnw1_tr = softmax_ln(inputs["ew_tr"])
    lnw0_te, lnw1_te = softmax_ln(inputs["ew_te"])
    lfs_tr = np.asarray(inputs["lfs_tr"]).astype(f64)
    lfs_te = np.asarray(inputs["lfs_te"]).astype(f64)

    in_maps = []
    for core in range(NCORES):
        ntr = slice(core * NTRL, (core + 1) * NTRL)
        nte = slice(core * NTEL, (core + 1) * NTEL)
        # pack [7, N, 4] -> per-block lhsT [7, 4 blocks * (4i x 32n)]
        def pack_tr(allc):
            cols = np.empty((7, 512), np.float16)
            sub = allc[:, ntr, :]                      # [7, 128, 4]
            for b in range(4):
                blk = sub[:, b * 32:(b + 1) * 32, :]   # [7, 32n, 4i]
                cols[:, b * 128:(b + 1) * 128] = (
                    blk.transpose(0, 2, 1).reshape(7, 128))
            return cols

        def pack_te(allc):
            blk = allc[:, nte, :]                      # [7, 32, 4]
            return blk.transpose(0, 2, 1).reshape(7, 128).copy()

        ltor_te = np.zeros((8, 512), np.float16)
        for a in range(4):
            ltor_te[:, a * 128 + a * 32: a * 128 + (a + 1) * 32] = \
                tor_te_all[:, nte]

        b_tr1 = np.float32(lnw1_tr[ntr] + lfs_tr[ntr]).reshape(128, 1)
        b_tr0 = np.float32(lnw0_tr[ntr] + lfs_tr[ntr]).reshape(128, 1)
        b_te1 = np.tile(np.float32(lnw1_te[nte] + lfs_te[nte]), 4).reshape(128, 1)
        b_te0 = np.tile(np.float32(lnw0_te[nte] + lfs_te[nte]), 4).reshape(128, 1)
        c0s = np.full((128, 1), np.float32(coeff0), np.float32)

        im = dict(shared)
        im.update(lu_tr=pack_tr(lu_tr_all), lv_tr=pack_tr(lv_tr_all),
                  lu_te=pack_te(lu_te_all), lv_te=pack_te(lv_te_all),
                  ltor_tr=tor_tr_all[:, ntr].copy().astype(np.float16),
                  ltor_te=ltor_te,
                  b_tr1=b_tr1, b_tr0=b_tr0, b_te1=b_te1, b_te0=b_te0, c0s=c0s)
        in_maps.append(im)
    return in_maps


def _assemble(results):
    out_tr = np.empty((B, NTR, T), np.float32)
    out_te = np.empty((B, NTE, T), np.float32)
    for core in range(NCORES):
        dev = results[core]["out_tr"]            # [128, BT]
        out_tr[:, core * NTRL:(core + 1) * NTRL, :] = (
            dev.reshape(NTRL, B, T).transpose(1, 0, 2))
        devt = results[core]["out_te"]           # [128, 1024]
        for q in range(NQUAD):
            for a in range(4):
                blk = devt[a * 32:(a + 1) * 32, q * CHUNK:(q + 1) * CHUNK]
                bt0 = q * 4 * CHUNK + a * CHUNK
                b0 = bt0 // T
                nb = CHUNK // T
                out_te[b0:b0 + nb, core * NTEL:(core + 1) * NTEL, :] = (
                    blk.reshape(NTEL, nb, T).transpose(1, 0, 2))
    return out_tr, out_te


def kernel(**inputs):
    nc = _build()
    in_maps = _prepare(inputs)
    res = run_bass_kernel_spmd(nc, in_maps, core_ids=list(range(NCORES)))
    return _assemble(res.results)


# revision 6
# speedup vs baseline: 1.0286x; 1.0286x over previous
"""Trainium2 Bass kernel for the Gaussian-bump decoder (nn_Decoder).

Math (per (bt, n) with bt = b*T + t):
  torus ensemble (K=1, L=1):
    g0   = exp(-((sin z0 - sin m0 + sin rf0_n)^2 + (cos z0 - cos m0 + cos rf0_n)^2)/s0^2)
    r0   = exp(coeff0 * g0)
  euclidean ensemble (K=16 on a separable 4x4 grid, L=2):
    U_i  = exp(-(z1_0 + rf1_n0 - p_i)^2 / s^2)
    V_j  = exp(-(z1_1 + rf1_n1 - p_j)^2 / s^2)
    resp = sum_ij C_ij U_i V_j
    r1   = exp(resp)
  out  = (w0_n r0 + w1_n r1) * exp(lfs_n)   -> returned as (B, N, T)

Device mapping (per core, neurons sharded 8 ways: 128 train + 32 test):
  - All exp-args are tiny-contraction fp16 matmuls (hi/lo split => ~fp32
    accuracy) with the z-derived "moving" rows built on host.
  - U/V tiles are packed [128 = 4i x 32n, bt-chunk]; the 4x4 coefficient
    contraction (W = C V) and the i-sum are fp16 matmuls with block
    patterns; the softmax weights / lfs fold into the exp biases.
"""

import numpy as np

import concourse.bass as bass
import concourse.tile as tile
from concourse import bacc, mybir
from concourse.bass_utils import run_bass_kernel_spmd

F32 = mybir.dt.float32
F16 = mybir.dt.float16
EXP = mybir.ActivationFunctionType.Exp
MULT = mybir.AluOpType.mult
ADD = mybir.AluOpType.add

B, T = 16, 256
NTR, NTE = 1024, 256
BT = B * T                     # 4096
NCORES = 8
NTRL = NTR // NCORES           # 128 train neurons per core
NTEL = NTE // NCORES           # 32 test neurons per core
GK = 4                         # grid side (K = GK*GK)
CHUNK = 512
NCHUNK = BT // CHUNK           # 8
NQUAD = NCHUNK // 4            # 2 packed quads for the test ensemble

_DRAM_SPECS = [
    ("m1u", [7, BT], F16), ("m1v", [7, BT], F16), ("m0", [8, BT], F16),
    ("cbdh", [128, 128], F16), ("cbdl", [128, 128], F16),
    ("sumpat", [128, 512], F16),
    ("lu_tr", [7, 512], F16), ("lv_tr", [7, 512], F16),
    ("lu_te", [7, 128], F16), ("lv_te", [7, 128], F16),
    ("ltor_tr", [8, 128], F16), ("ltor_te", [8, 512], F16),
    ("b_tr1", [128, 1], F32), ("b_tr0", [128, 1], F32),
    ("b_te1", [128, 1], F32), ("b_te0", [128, 1], F32),
    ("c0s", [128, 1], F32),
]


def _emit(tc, d):
    nc = tc.nc
    with (
        tc.tile_pool(name="consts", bufs=1) as cp,
        tc.tile_pool(name="pu", bufs=1, space="PSUM") as pup,
        tc.tile_pool(name="pv", bufs=1, space="PSUM") as pvp,
        tc.tile_pool(name="pw", bufs=2, space="PSUM") as pwp,
        tc.tile_pool(name="presp", bufs=1, space="PSUM") as prp,
        tc.tile_pool(name="pg0", bufs=1, space="PSUM") as pgp,
        tc.tile_pool(name="pte", bufs=1, space="PSUM") as ptep,
        tc.tile_pool(name="sb", bufs=3) as sp,
        tc.tile_pool(name="outp", bufs=3) as op_,
    ):
        # ---- preload the exp table set before any data arrives ----
        warm = cp.tile([1, 2], F32, tag="warm", name="warm")
        nc.vector.memset(warm[0:1, 0:1], 0.0)
        nc.scalar.activation(warm[0:1, 1:2], warm[0:1, 0:1], EXP)

        # ---- load constants (matmul-critical tensors first) ----
        ct = {}
        order = ["lu_tr", "m1u", "lv_tr", "m1v", "lu_te", "lv_te", "cbdh",
                 "cbdl", "sumpat", "ltor_tr", "ltor_te", "m0"]
        specs = {name: (shape, dt) for name, shape, dt in _DRAM_SPECS}
        names = order + [n for n in specs if n not in order]
        for name in names:
            shape, dt = specs[name]
            ct[name] = cp.tile(shape, dt, tag=name, name=name)
            nc.sync.dma_start(ct[name][:], d[name][:])

        def act(out_ap, in_ap, scale=1.0, bias=0.0):
            nc.scalar.activation(out_ap, in_ap, EXP, scale=scale, bias=bias)

        pte_resp = pte_g0 = None
        for c in range(NCHUNK):
            sl = bass.ts(c, CHUNK)
            q, a = divmod(c, 4)

            # ---------------- test (chunk a of quad q) ----------------
            if a == 0:
                pte_resp = ptep.tile([128, CHUNK], F32, tag="pte_resp")
                pte_g0 = ptep.tile([128, CHUNK], F32, tag="pte_g0")
            pu = pup.tile([128, CHUNK], F32, tag="pu")
            nc.tensor.matmul(pu[:], ct["lu_te"][:], ct["m1u"][:, sl],
                             start=True, stop=True)
            usb = sp.tile([128, CHUNK], F32, tag="usb")
            act(usb[:], pu[:])
            pv = pvp.tile([128, CHUNK], F32, tag="pv")
            nc.tensor.matmul(pv[:], ct["lv_te"][:], ct["m1v"][:, sl],
                             start=True, stop=True)
            vsb = sp.tile([128, CHUNK], F16, tag="vsb")
            act(vsb[:], pv[:])
            pw = pwp.tile([128, CHUNK], F32, tag="pw")
            nc.tensor.matmul(pw[:], ct["cbdh"][:], vsb[:], start=True, stop=False)
            nc.tensor.matmul(pw[:], ct["cbdl"][:], vsb[:], start=False, stop=True)
            ssb = sp.tile([128, CHUNK], F16, tag="ssb")
            nc.vector.tensor_tensor(ssb[:], usb[:], pw[:], op=MULT)
            asl = bass.ts(a, 128)
            nc.tensor.matmul(pte_resp[:], ct["sumpat"][:, asl], ssb[:],
                             start=(a == 0), stop=(a == 3))
            nc.tensor.matmul(pte_g0[:], ct["ltor_te"][:, asl], ct["m0"][:, sl],
                             start=(a == 0), stop=(a == 3))
            if a == 3:
                r1te = sp.tile([128, CHUNK], F32, tag="r1te")
                act(r1te[:], pte_resp[:], bias=ct["b_te1"][:, 0:1])
                g0te = sp.tile([128, CHUNK], F32, tag="g0te")
                act(g0te[:], pte_g0[:])
                r0te = sp.tile([128, CHUNK], F32, tag="r0te")
                act(r0te[:], g0te[:], scale=ct["c0s"][:, 0:1], bias=ct["b_te0"][:, 0:1])
                ote = op_.tile([128, CHUNK], F32, tag="ote")
                nc.vector.tensor_tensor(ote[:], r0te[:], r1te[:], op=ADD)
                nc.sync.dma_start(d["out_te"][:, bass.ts(q, CHUNK)], ote[:])

            # ---------------- train: euclidean ensemble ----------------
            presp = prp.tile([128, CHUNK], F32, tag="presp")
            for blk in range(4):
                bsl = bass.ts(blk, 128)
                pu = pup.tile([128, CHUNK], F32, tag="pu")
                nc.tensor.matmul(pu[:], ct["lu_tr"][:, bsl], ct["m1u"][:, sl],
                                 start=True, stop=True)
                usb = sp.tile([128, CHUNK], F32, tag="usb")
                act(usb[:], pu[:])
                pv = pvp.tile([128, CHUNK], F32, tag="pv")
                nc.tensor.matmul(pv[:], ct["lv_tr"][:, bsl], ct["m1v"][:, sl],
                                 start=True, stop=True)
                vsb = sp.tile([128, CHUNK], F16, tag="vsb")
                act(vsb[:], pv[:])
                pw = pwp.tile([128, CHUNK], F32, tag="pw")
                nc.tensor.matmul(pw[:], ct["cbdh"][:], vsb[:], start=True, stop=False)
                nc.tensor.matmul(pw[:], ct["cbdl"][:], vsb[:], start=False, stop=True)
                ssb = sp.tile([128, CHUNK], F16, tag="ssb")
                nc.vector.tensor_tensor(ssb[:], usb[:], pw[:], op=MULT)
                nc.tensor.matmul(presp[:], ct["sumpat"][:, bsl], ssb[:],
                                 start=(blk == 0), stop=(blk == 3))
            r1 = sp.tile([128, CHUNK], F32, tag="r1")
            act(r1[:], presp[:], bias=ct["b_tr1"][:, 0:1])

            # ---------------- train: torus ensemble ----------------
            pg0 = pgp.tile([128, CHUNK], F32, tag="pg0")
            nc.tensor.matmul(pg0[:], ct["ltor_tr"][:], ct["m0"][:, sl],
                             start=True, stop=True)
            g0 = sp.tile([128, CHUNK], F32, tag="g0")
            act(g0[:], pg0[:])
            r0 = sp.tile([128, CHUNK], F32, tag="r0")
            act(r0[:], g0[:], scale=ct["c0s"][:, 0:1], bias=ct["b_tr0"][:, 0:1])

            otr = op_.tile([128, CHUNK], F32, tag="otr")
            nc.vector.tensor_tensor(otr[:], r0[:], r1[:], op=ADD)
            nc.sync.dma_start(d["out_tr"][:, sl], otr[:])


_CACHE = {}


def _build():
    if "nc" in _CACHE:
        return _CACHE["nc"]
    nc = bacc.Bacc("TRN2", target_bir_lowering=False, debug=False,
                   num_devices=NCORES)
    d = {}
    for name, shape, dt in _DRAM_SPECS:
        d[name] = nc.dram_tensor(name, shape, dt, kind="ExternalInput").ap()
    d["out_tr"] = nc.dram_tensor("out_tr", [NTRL, BT], F32,
                                 kind="ExternalOutput").ap()
    d["out_te"] = nc.dram_tensor("out_te", [128, NQUAD * CHUNK], F32,
                                 kind="ExternalOutput").ap()
    with tile.TileContext(nc) as tc:
        _emit(tc, d)
    nc.compile()
    _CACHE["nc"] = nc
    return nc


def _hl(x):
    """fp32 -> (hi, lo) fp16 split."""
    x = np.asarray(x, np.float32)
    h = x.astype(np.float16)
    lo = (x - h.astype(np.float32)).astype(np.float16)
    return h, lo


def _arg_consts(rf_col, pts, s2):
    """lhsT columns for  -(z + rf - p_i)^2 / s2  per (neuron, i).

    Returns [7, N, 4] fp16: rows pair with moving rows
    [qh, ql, zh, zl, zh, 1, 1] where q = -z^2/s2.
    """
    c = rf_col[:, None] - pts[None, :]          # (N, 4) fp64
    g = np.float32(-2.0 * c / s2)
    b = np.float32(-(c * c) / s2)
    gh, gl = _hl(g)
    bh, bl = _hl(b)
    ones = np.ones_like(gh)
    return np.stack([ones, ones, gh, gh, gl, bh, bl]).astype(np.float16)


def _moving_rows(z_col, s2):
    """Moving rows [qh, ql, zh, zl, zh, 1, 1] with q = -z^2/s2."""
    q = np.float32(-(z_col.astype(np.float64) ** 2) / s2)
    qh, ql = _hl(q)
    zh, zl = _hl(np.float32(z_col))
    ones = np.ones(z_col.shape[0], np.float32)
    return np.stack([qh, ql, zh, zl, zh, ones, ones]).astype(np.float16)


def _prepare(inputs):
    f64 = np.float64
    z0 = np.asarray(inputs["z0"]).reshape(BT).astype(f64)
    z1 = np.asarray(inputs["z1"]).reshape(BT, 2).astype(f64)
    coeff0 = f64(np.asarray(inputs["coeff0"])[0, 0])
    mean0 = f64(np.asarray(inputs["mean0"]).reshape(-1)[0])
    log_var0 = f64(np.asarray(inputs["log_var0"]).reshape(-1)[0])
    C = np.asarray(inputs["coeff1"]).reshape(GK, GK).astype(f64)
    mean1 = np.asarray(inputs["mean1"]).reshape(GK * GK, 2).astype(f64)
    log_var1 = np.asarray(inputs["log_var1"]).reshape(GK * GK, 2).astype(f64)
    # separable grid: mean1[(i,j), 0] = pts[i], mean1[(i,j), 1] = pts[j]
    pts0 = mean1.reshape(GK, GK, 2)[:, 0, 0]
    pts1 = mean1.reshape(GK, GK, 2)[0, :, 1]
    s2u = f64(np.exp(log_var1[0, 0])) ** 2
    s2v = f64(np.exp(log_var1[0, 1])) ** 2
    s20 = f64(np.exp(log_var0)) ** 2

    # shared moving operands
    m1u = _moving_rows(z1[:, 0], s2u)
    m1v = _moving_rows(z1[:, 1], s2v)
    as_ = np.float32(np.sin(z0) - np.sin(mean0))
    ac_ = np.float32(np.cos(z0) - np.cos(mean0))
    p = np.float32(-(as_.astype(f64) ** 2 + ac_.astype(f64) ** 2 + 1.0) / s20)
    ph, pl = _hl(p)
    ash, asl = _hl(as_)
    ach, acl = _hl(ac_)
    m0 = np.stack([ph, pl, ash, asl, ash, ach, acl, ach]).astype(np.float16)

    # C block-diagonal (4x4 per 32-neuron diag block), partitions i-major:
    # row p_in = j*32 + n32 ; col p_out = i*32 + n32 ; value C[i, j]
    Ch, Cl = _hl(np.float32(C))
    cbdh = np.zeros((128, 128), np.float16)
    cbdl = np.zeros((128, 128), np.float16)
    for i in range(GK):
        for j in range(GK):
            idx = np.arange(32)
            cbdh[j * 32 + idx, i * 32 + idx] = Ch[i, j]
            cbdl[j * 32 + idx, i * 32 + idx] = Cl[i, j]
    # i-sum patterns: block b: lhsT[p, col] = 1 iff col == 32*b + (p % 32)
    sumpat = np.zeros((128, 512), np.float16)
    for b in range(4):
        for i in range(GK):
            idx = np.arange(32)
            sumpat[i * 32 + idx, 128 * b + 32 * b + idx] = 1.0
    shared = dict(m1u=m1u, m1v=m1v, m0=m0, cbdh=cbdh, cbdl=cbdl, sumpat=sumpat)

    rf_tr0 = np.asarray(inputs["rf_tr0"]).reshape(NTR).astype(f64)
    rf_te0 = np.asarray(inputs["rf_te0"]).reshape(NTE).astype(f64)
    rf_tr1 = np.asarray(inputs["rf_tr1"]).astype(f64)
    rf_te1 = np.asarray(inputs["rf_te1"]).astype(f64)
    lu_tr_all = _arg_consts(rf_tr1[:, 0], pts0, s2u)   # [7, NTR, 4]
    lv_tr_all = _arg_consts(rf_tr1[:, 1], pts1, s2v)
    lu_te_all = _arg_consts(rf_te1[:, 0], pts0, s2u)
    lv_te_all = _arg_consts(rf_te1[:, 1], pts1, s2v)

    def torus_consts(rf0):
        bs = np.float32(np.sin(rf0))
        bc = np.float32(np.cos(rf0))
        gs = np.float32(-2.0 * bs.astype(f64) / s20)
        gc = np.float32(-2.0 * bc.astype(f64) / s20)
        gsh, gsl = _hl(gs)
        gch, gcl = _hl(gc)
        ones = np.ones_like(gsh)
        return np.stack([ones, ones, gsh, gsh, gsl, gch, gch, gcl]).astype(np.float16)

    tor_tr_all = torus_consts(rf_tr0)   # [8, NTR]
    tor_te_all = torus_consts(rf_te0)   # [8, NTE]

    def softmax_ln(ew):
        ew = np.asarray(ew).astype(f64)
        m = ew.max(axis=1, keepdims=True)
        e = np.exp(ew - m)
        w = e / e.sum(axis=1, keepdims=True)
        return np.log(w[:, 0]), np.log(w[:, 1])

    lnw0_tr, lnw1_tr = softmax_ln(inputs["ew_tr"])
    lnw0_te, lnw1_te = softmax_ln(inputs["ew_te"])
    lfs_tr = np.asarray(inputs["lfs_tr"]).astype(f64)
    lfs_te = np.asarray(inputs["lfs_te"]).astype(f64)

    in_maps = []
    for core in range(NCORES):
        ntr = slice(core * NTRL, (core + 1) * NTRL)
        nte = slice(core * NTEL, (core + 1) * NTEL)
        # pack [7, N, 4] -> per-block lhsT [7, 4 blocks * (4i x 32n)]
        def pack_tr(allc):
            cols = np.empty((7, 512), np.float16)
            sub = allc[:, ntr, :]                      # [7, 128, 4]
            for b in range(4):
                blk = sub[:, b * 32:(b + 1) * 32, :]   # [7, 32n, 4i]
                cols[:, b * 128:(b + 1) * 128] = (
                    blk.transpose(0, 2, 1).reshape(7, 128))
            return cols

        def pack_te(allc):
            blk = allc[:, nte, :]                      # [7, 32, 4]
            return blk.transpose(0, 2, 1).reshape(7, 128).copy()

        ltor_te = np.zeros((8, 512), np.float16)
        for a in range(4):
            ltor_te[:, a * 128 + a * 32: a * 128 + (a + 1) * 32] = \
                tor_te_all[:, nte]

        b_tr1 = np.float32(lnw1_tr[ntr] + lfs_tr[ntr]).reshape(128, 1)
        b_tr0 = np.float32(lnw0_tr[ntr] + lfs_tr[ntr]).reshape(128, 1)
        b_te1 = np.tile(np.float32(lnw1_te[nte] + lfs_te[nte]), 4).reshape(128, 1)
        b_te0 = np.tile(np.float32(lnw0_te[nte] + lfs_te[nte]), 4).reshape(128, 1)
        c0s = np.full((128, 1), np.float32(coeff0), np.float32)

        im = dict(shared)
        im.update(lu_tr=pack_tr(lu_tr_all), lv_tr=pack_tr(lv_tr_all),
                  lu_te=pack_te(lu_te_all), lv_te=pack_te(lv_te_all),
                  ltor_tr=tor_tr_all[:, ntr].copy().astype(np.float16),
                  ltor_te=ltor_te,
                  b_tr1=b_tr1, b_tr0=b_tr0, b_te1=b_te1, b_te0=b_te0, c0s=c0s)
        in_maps.append(im)
    return in_maps


def _assemble(results):
    out_tr = np.empty((B, NTR, T), np.float32)
    out_te = np.empty((B, NTE, T), np.float32)
    for core in range(NCORES):
        dev = results[core]["out_tr"]            # [128, BT]
        out_tr[:, core * NTRL:(core + 1) * NTRL, :] = (
            dev.reshape(NTRL, B, T).transpose(1, 0, 2))
        devt = results[core]["out_te"]           # [128, 1024]
        for q in range(NQUAD):
            for a in range(4):
                blk = devt[a * 32:(a + 1) * 32, q * CHUNK:(q + 1) * CHUNK]
                bt0 = q * 4 * CHUNK + a * CHUNK
                b0 = bt0 // T
                nb = CHUNK // T
                out_te[b0:b0 + nb, core * NTEL:(core + 1) * NTEL, :] = (
                    blk.reshape(NTEL, nb, T).transpose(1, 0, 2))
    return out_tr, out_te


def kernel(**inputs):
    nc = _build()
    in_maps = _prepare(inputs)
    res = run_bass_kernel_spmd(nc, in_maps, core_ids=list(range(NCORES)))
    return _assemble(res.results)
